# revision 18
# baseline (speedup 1.0000x reference)
"""Two-layer GATConv (PyG-style, edge_dim=1, add_self_loops fill='mean') on
8 trn2 NeuronCores.

Strategy (v3: project-once, gather-h, contiguous-inner layouts)
---------------------------------------------------------------
Destinations are partitioned across the 8 cores (degree-sorted, dealt
round-robin).  Three device programs per kernel call:

  P1  per-node projection h1 = x @ [W1 | W1@A_src1 | W1@A_dst1]
      (bf16 matmul, one persistent xT load, grouped output DMAs).
  P2  layer-1 edge aggregation over a [128 dst x K_t slot] grid whose
      slot payloads are HOST-GATHERED h1 rows (bf16, hc-major k-inner
      blocks).  alpha on Pool, leaky+exp(+Z via accum_out) on ACT,
      normalized-attention product on DVE (all-bf16), bf16 fold-halving
      + f32 segment-reduce on DVE, bias on Pool, relu on ACT, fused
      layer-2 projection on PE.  Outputs h2 rows.
  P3  layer-2 edge aggregation, same grid, slots gathered from h2.

All FLOPs run on device; the host only moves data (gather/scatter of
rows, dtype rounding).
"""
import copy
import os

import numpy as np
import ml_dtypes

import concourse.bass as bass
import concourse.mybir as mybir
import concourse.tile as tile
from contextlib import ExitStack
from concourse.bass_utils import run_bass_kernel_spmd

NCORES = 8
P = 128
N = 50000
E = 800000
NPC = N // NCORES            # 6250 dsts per core
T = (NPC + P - 1) // P       # 49 tiles
NROWS = T * P                # 6272 rows per core (incl pad dsts)
G = 7                        # tiles per output-DMA group (49 = 7*7)
NEG_SLOPE = 0.2

F32 = mybir.dt.float32
BF16 = mybir.dt.bfloat16
NPBF = ml_dtypes.bfloat16

LAST_EXEC_NS = []


# --------------------------------------------------------------------------
# walrus workaround: cap sync waits per instruction (see v1 notes)
# --------------------------------------------------------------------------
def _split_waits(nc, limit=1):
    sem = nc.alloc_semaphore("wsplit_tmpl_sem")
    tmpl = {}
    for eng_ty, eng in nc.engines.items():
        tmpl[eng_ty] = eng.wait_ge(sem, 0).ins
    tmpl_names = {mi.name for mi in tmpl.values()}
    for f in nc.m.functions:
        for bb in f.blocks:
            insts = [i for i in bb.instructions if i.name not in tmpl_names]
            out = []
            for inst in insts:
                si = inst.sync_info
                waits = list(si.on_wait) if si and si.on_wait else []
                tn = type(inst).__name__
                eff = 0 if (tn == "InstDrain" or "Branch" in tn) else limit
                if len(waits) > eff:
                    head = waits[:-eff] if eff else waits
                    for w in head:
                        c = copy.deepcopy(tmpl[inst.engine])
                        c.name = f"I-wsplit-{nc.next_id()}"
                        c.sync_info = mybir.SyncInfo(on_wait=[w], on_update=[])
                        out.append(c)
                    inst.sync_info = mybir.SyncInfo(
                        on_wait=waits[-eff:] if eff else [],
                        on_update=list(si.on_update) if si.on_update else [],
                    )
                out.append(inst)
            bb.instructions = out


def _ap(root, extra_off, dims):
    return bass.AP(root.tensor, root.offset + extra_off, [list(d) for d in dims])


# --------------------------------------------------------------------------
# P1: per-node projection  h = x @ Wcat   (Wcat = [W | Wa_src | Wa_dst])
# --------------------------------------------------------------------------
def _build_proj(COLS, HC):
    nc = bass.Bass()
    xT = nc.dram_tensor("xT", [P, NROWS], BF16, kind="ExternalInput")
    wcat = nc.dram_tensor("wcat", [P, COLS], BF16, kind="ExternalInput")
    hb = nc.dram_tensor("hb", [NROWS, COLS], F32, kind="ExternalOutput")

    with ExitStack() as ctx:
        tc = ctx.enter_context(tile.TileContext(nc))
        pers = ctx.enter_context(tc.tile_pool(name="pers", bufs=1))
        sb = ctx.enter_context(tc.tile_pool(name="sb", bufs=2))
        ps = ctx.enter_context(tc.tile_pool(name="ps", bufs=4, space="PSUM"))

        wc = pers.tile([P, COLS], BF16)
        nc.sync.dma_start(out=wc[:], in_=wcat[:, :])
        xa = pers.tile([P, NROWS], BF16)
        nc.sync.dma_start(out=xa[:], in_=xT[:, :])

        hb_root = hb[:, :]
        for g in range(T // G):
            HG = sb.tile([P, G * COLS], F32, tag="HG")
            for j in range(G):
                t = g * G + j
                pg = ps.tile([P, COLS], F32, tag="pg")
                nc.tensor.matmul(out=pg[:], lhsT=xa[:, t * P:(t + 1) * P],
                                 rhs=wc[:], start=True, stop=True)
                nc.scalar.copy(out=HG[:, j * COLS:(j + 1) * COLS], in_=pg[:])
            hb_ap = _ap(hb_root, g * G * P * COLS,
                        [(COLS, P), (P * COLS, G), (1, COLS)])
            nc.sync.dma_start(out=hb_ap, in_=HG[:])

    _split_waits(nc)
    return nc


# --------------------------------------------------------------------------
# P2/P3: edge aggregation over the slot grid (slot payload = gathered h)
# --------------------------------------------------------------------------
def _build_agg(KT, H, C, relu, proj_cols):
    """hs blocks per tile: [HC, K_t] (hc-major, k contiguous).
    asr: [P, H*SK] (h-major).  ads: [P, H*T]."""
    HC = H * C
    SK = sum(KT)
    OUTC = proj_cols if proj_cols else HC

    nc = bass.Bass()
    hs = nc.dram_tensor("hs", [P, SK * HC], BF16, kind="ExternalInput")
    asr = nc.dram_tensor("asr", [P, H * SK], F32, kind="ExternalInput")
    ads = nc.dram_tensor("ads", [P, H * T], F32, kind="ExternalInput")
    warr = nc.dram_tensor("warr", [P, SK], F32, kind="ExternalInput")
    invc = nc.dram_tensor("invc", [P, T], F32, kind="ExternalInput")
    kk = nc.dram_tensor("kk", [P, H], F32, kind="ExternalInput")
    bvec = nc.dram_tensor("bvec", [P, HC], F32, kind="ExternalInput")
    if proj_cols:
        w2c = nc.dram_tensor("w2c", [P, proj_cols], BF16, kind="ExternalInput")
        idt = nc.dram_tensor("idt", [P, P], BF16, kind="ExternalInput")
    outp = nc.dram_tensor("out", [NROWS, OUTC], F32, kind="ExternalOutput")

    with ExitStack() as ctx:
        tc = ctx.enter_context(tile.TileContext(nc))
        pers = ctx.enter_context(tc.tile_pool(name="pers", bufs=1))
        hp = ctx.enter_context(tc.tile_pool(name="hp", bufs=3))
        sb = ctx.enter_context(tc.tile_pool(name="sb", bufs=3))
        og = ctx.enter_context(tc.tile_pool(name="og", bufs=2))
        if proj_cols:
            ps = ctx.enter_context(tc.tile_pool(name="ps", bufs=3, space="PSUM"))

        asr_t = pers.tile([P, H * SK], F32)
        nc.sync.dma_start(out=asr_t[:], in_=asr[:, :])
        ads_t = pers.tile([P, H * T], F32)
        nc.sync.dma_start(out=ads_t[:], in_=ads[:, :])
        wall = pers.tile([P, SK], F32)
        nc.sync.dma_start(out=wall[:], in_=warr[:, :])
        iva = pers.tile([P, T], F32)
        nc.sync.dma_start(out=iva[:], in_=invc[:, :])
        kt = pers.tile([P, H], F32)
        nc.sync.dma_start(out=kt[:], in_=kk[:, :])
        bt = pers.tile([P, HC], F32)
        nc.sync.dma_start(out=bt[:], in_=bvec[:, :])
        if proj_cols:
            w2t = pers.tile([P, proj_cols], BF16)
            nc.sync.dma_start(out=w2t[:], in_=w2c[:, :])
            idtt = pers.tile([P, P], BF16)
            nc.sync.dma_start(out=idtt[:], in_=idt[:, :])

        kpitch = kt[:].ap[0][0]
        aspitch = asr_t[:].ap[0][0]
        adpitch = ads_t[:].ap[0][0]
        out_root = outp[:, :]
        cb = 0
        OG = None
        for t in range(T):
            K = KT[t]
            j = t % G
            if j == 0:
                OG = og.tile([P, G * OUTC], F32, tag="OG")
            HS = hp.tile([P, K * HC], BF16, tag="HS")
            nc.sync.dma_start(out=HS[:], in_=hs[:, cb * HC:(cb + K) * HC])

            # alpha[p, h, k] = a_src[slot] + a_dst[dst] + w*k_h   (Pool)
            A = sb.tile([P, H * K], F32, tag="A")
            a0 = A[:]
            apitch = a0.ap[0][0]
            A3 = _ap(a0, 0, [(apitch, P), (K, H), (1, K)])
            asrc_b = _ap(asr_t[:], cb, [(aspitch, P), (SK, H), (1, K)])
            adst_b = _ap(ads_t[:], t, [(adpitch, P), (T, H), (0, K)])
            nc.gpsimd.tensor_tensor(out=A3, in0=asrc_b, in1=adst_b,
                                    op=mybir.AluOpType.add)
            wt0 = wall[:, cb:cb + K]
            wpitch = wt0.ap[0][0]
            WK = sb.tile([P, H * K], F32, tag="WK")
            WK3 = _ap(WK[:], 0, [(WK[:].ap[0][0], P), (K, H), (1, K)])
            w_b = _ap(wt0, 0, [(wpitch, P), (0, H), (1, K)])
            kk_b = _ap(kt[:], 0, [(kpitch, P), (1, H), (0, K)])
            nc.gpsimd.tensor_tensor(out=WK3, in0=w_b, in1=kk_b,
                                    op=mybir.AluOpType.mult)
            nc.gpsimd.tensor_tensor(out=A3, in0=A3, in1=WK3,
                                    op=mybir.AluOpType.add)
            # self-loop alpha correction at k = K-1
            LA = sb.tile([P, 1], F32, tag="LA")
            nc.vector.tensor_reduce(out=LA[:], in_=wt0,
                                    axis=mybir.AxisListType.X,
                                    op=mybir.AluOpType.add)
            nc.gpsimd.tensor_tensor(out=LA[:], in0=LA[:], in1=iva[:, t:t + 1],
                                    op=mybir.AluOpType.mult)
            A_self = _ap(a0, K - 1, [(apitch, P), (K, H)])
            kk_b2 = _ap(kt[:], 0, [(kpitch, P), (1, H)])
            nc.vector.scalar_tensor_tensor(out=A_self, in0=kk_b2,
                                           scalar=LA[:], in1=A_self,
                                           op0=mybir.AluOpType.mult,
                                           op1=mybir.AluOpType.add)
            # leaky relu (DVE STT) + exp (ACT); Z via accum_out
            AL = sb.tile([P, H * K], F32, tag="AL")
            nc.vector.scalar_tensor_tensor(out=AL[:], in0=A[:],
                                           scalar=NEG_SLOPE, in1=A[:],
                                           op0=mybir.AluOpType.mult,
                                           op1=mybir.AluOpType.max)
            PP = sb.tile([P, H * K], BF16, tag="PP")
            Z = sb.tile([P, H], F32, tag="Z")
            for h in range(H):
                nc.scalar.activation(out=PP[:, h * K:(h + 1) * K],
                                     in_=AL[:, h * K:(h + 1) * K],
                                     func=mybir.ActivationFunctionType.Exp,
                                     accum_out=Z[:, h:h + 1])
            Zr = sb.tile([P, H], F32, tag="Zr")
            nc.vector.reciprocal(out=Zr[:], in_=Z[:])
            PPn = sb.tile([P, H * K], BF16, tag="PPn")
            pn0 = PPn[:]
            pnpitch = pn0.ap[0][0]
            Zr_b = _ap(Zr[:], 0, [(Zr[:].ap[0][0], P), (1, H), (0, K)])
            PP3 = _ap(PP[:], 0, [(PP[:].ap[0][0], P), (K, H), (1, K)])
            PPn3 = _ap(pn0, 0, [(pnpitch, P), (K, H), (1, K)])
            nc.vector.tensor_tensor(out=PPn3, in0=PP3, in1=Zr_b,
                                    op=mybir.AluOpType.mult)
            # PROD[p, k, hc] = HS[p, k, hc] * PPn[p, h, k]  (k-major; DVE)
            PROD = hp.tile([P, K * HC], BF16, tag="PROD")
            p0 = PROD[:]
            ppitch = p0.ap[0][0]
            h0 = HS[:]
            hpitch = h0.ap[0][0]
            PROD4 = _ap(p0, 0, [(ppitch, P), (C, H), (HC, K), (1, C)])
            HS4 = _ap(h0, 0, [(hpitch, P), (C, H), (HC, K), (1, C)])
            PPn_b = _ap(pn0, 0, [(pnpitch, P), (K, H), (1, K), (0, C)])
            nc.vector.tensor_tensor(out=PROD4, in0=HS4, in1=PPn_b,
                                    op=mybir.AluOpType.mult)
            # one f32 fold (chunk k + chunk k+nf): fully contiguous halves,
            # split Pool/DVE by element range; then strided f32 reduce (DVE)
            nf = K // 2
            FL = nf * HC
            F1 = hp.tile([P, FL], F32, tag="F1")
            f0 = F1[:]
            fpitch = f0.ap[0][0]
            s = (FL * 7 // 8) & ~63         # Pool handles [0, s)
            for eng, lo, hi in ((nc.gpsimd, 0, s), (nc.vector, s, FL)):
                dstap = _ap(f0, lo, [(fpitch, P), (1, hi - lo)])
                s0ap = _ap(p0, lo, [(ppitch, P), (1, hi - lo)])
                s1ap = _ap(p0, FL + lo, [(ppitch, P), (1, hi - lo)])
                eng.tensor_tensor(out=dstap, in0=s0ap, in1=s1ap,
                                  op=mybir.AluOpType.add)
            O = sb.tile([P, HC], F32, tag="O")
            Or = _ap(O[:], 0, [(O[:].ap[0][0], P), (1, HC)])
            F1r = _ap(f0, 0, [(fpitch, P), (1, HC), (HC, nf)])
            nc.vector.tensor_reduce(out=Or, in_=F1r,
                                    axis=mybir.AxisListType.X,
                                    op=mybir.AluOpType.add)
            # + bias (Pool); b is zero in this workload but kept general
            if proj_cols:
                Ob = sb.tile([P, HC], F32, tag="Ob")
                nc.gpsimd.tensor_tensor(out=Ob[:], in0=O[:], in1=bt[:],
                                        op=mybir.AluOpType.add)
                R = sb.tile([P, HC], BF16, tag="R")
                nc.scalar.activation(out=R[:], in_=Ob[:],
                                     func=mybir.ActivationFunctionType.Relu)
                tp = ps.tile([P, P], BF16, tag="tp")
                nc.tensor.transpose(out=tp[:], in_=R[:], identity=idtt[:])
                rt = sb.tile([P, P], BF16, tag="rt")
                nc.scalar.copy(out=rt[:], in_=tp[:])
                h2p = ps.tile([P, proj_cols], F32, tag="h2p")
                nc.tensor.matmul(out=h2p[:], lhsT=rt[:], rhs=w2t[:],
                                 start=True, stop=True)
                nc.scalar.copy(out=OG[:, j * OUTC:(j + 1) * OUTC], in_=h2p[:])
            else:
                nc.gpsimd.tensor_tensor(out=OG[:, j * OUTC:(j + 1) * OUTC],
                                        in0=O[:], in1=bt[:],
                                        op=mybir.AluOpType.add)
            if j == G - 1:
                g0i = t - G + 1
                out_ap = _ap(out_root, g0i * P * OUTC,
                             [(OUTC, P), (P * OUTC, G), (1, OUTC)])
                nc.sync.dma_start(out=out_ap, in_=OG[:])
            cb += K

    _split_waits(nc)
    return nc


# --------------------------------------------------------------------------
# host-side planning (identical partition to baseline)
# --------------------------------------------------------------------------
def _plan(edge_index):
    src = np.asarray(edge_index[0], dtype=np.int64)
    dst = np.asarray(edge_index[1], dtype=np.int64)
    deg = np.bincount(dst, minlength=N)
    order = np.argsort(-deg, kind="stable")
    rank_of = np.empty(N, np.int64)
    rank_of[order] = np.arange(N)
    core_of = (rank_of % NCORES).astype(np.int64)
    loc_of = (rank_of // NCORES).astype(np.int64)

    KT = []
    for t in range(T):
        r0 = min(1024 * t, N - 1)
        k = int(deg[order[r0]]) + 1
        KT.append(k + (k & 1))            # even K -> clean fold pairing
    KT = [max(k, 2) for k in KT]
    cbs = np.concatenate([[0], np.cumsum(KT)])

    eorder = np.argsort(dst, kind="stable")
    starts = np.concatenate([[0], np.cumsum(deg)])
    kpos_sorted = np.arange(E) - starts[dst[eorder]]
    kpos = np.empty(E, np.int64)
    kpos[eorder] = kpos_sorted

    e_core = core_of[dst]
    e_loc = loc_of[dst]
    e_t = e_loc >> 7
    e_p = e_loc & 127
    e_scol = cbs[e_t] + kpos

    return dict(src=src, dst=dst, deg=deg, order=order, core_of=core_of,
                loc_of=loc_of, KT=KT, cbs=cbs, e_core=e_core, e_t=e_t,
                e_p=e_p, e_scol=e_scol, kpos=kpos)


def _gather_inputs(plan, hb_full, att_full, ew, kvec, bias, H, C, w2c=None):
    """Per-core input maps for one aggregation layer.
    hb_full: [N, H*C] bf16; att_full: [N, 2H] f32 ([a_src | a_dst])."""
    HC = H * C
    KT, cbs = plan["KT"], plan["cbs"]
    SK = int(cbs[-1])
    src, e_core = plan["src"], plan["e_core"]
    e_p, e_t, kpos = plan["e_p"], plan["e_t"], plan["kpos"]
    e_scol = plan["e_scol"]
    order, deg = plan["order"], plan["deg"]
    KTa = np.array(KT)

    maps = []
    for c in range(NCORES):
        m = e_core == c
        asr = np.full((P, H, SK), -1e4, np.float32)
        war = np.zeros((P, SK), np.float32)
        asr[e_p[m], :, e_scol[m]] = att_full[src[m], :H]
        war[e_p[m], e_scol[m]] = ew[m]
        nodes = order[c::NCORES]
        loc = np.arange(nodes.size)
        tt = loc >> 7
        pp = loc & 127
        self_col = cbs[tt] + KTa[tt] - 1
        asr[pp, :, self_col] = att_full[nodes, :H]
        ads = np.zeros((P, H, T), np.float32)
        ads[pp, :, tt] = att_full[nodes, H:]
        iv = np.ones((P, T), np.float32)
        iv[pp, tt] = 1.0 / np.maximum(deg[nodes], 1.0)
        # hs: per-tile [P, K_t, HC] blocks (k-major, hc inner)
        hsr = np.zeros((P, SK, HC), NPBF)
        hsr[e_p[m], e_scol[m]] = hb_full[src[m]]
        hsr[pp, self_col] = hb_full[nodes]
        mp = {
            "hs": hsr.reshape(P, SK * HC),
            "asr": np.ascontiguousarray(asr.reshape(P, H * SK)),
            "ads": np.ascontiguousarray(ads.reshape(P, H * T)),
            "warr": war,
            "invc": iv,
            "kk": np.tile(kvec.reshape(1, H).astype(np.float32), (P, 1)),
            "bvec": np.tile(bias.reshape(1, -1).astype(np.float32), (P, 1)),
        }
        if w2c is not None:
            mp["w2c"] = w2c
            mp["idt"] = np.eye(P, dtype=NPBF)
        maps.append(mp)
    return maps


def _collect(plan, results, key):
    stack = np.stack([np.asarray(r[key]) for r in results])
    return stack[plan["core_of"], plan["loc_of"], :]


def _wcat(W, att_src, att_dst, H, C):
    Wa_s = np.stack([W[:, h * C:(h + 1) * C] @ att_src[h] for h in range(H)], 1)
    Wa_d = np.stack([W[:, h * C:(h + 1) * C] @ att_dst[h] for h in range(H)], 1)
    return np.concatenate([W, Wa_s, Wa_d], axis=1).astype(np.float32)


def kernel(x, edge_index, edge_weight, W1, att_src1, att_dst1, W_edge1,
           att_edge1, b1, W2, att_src2, att_dst2, W_edge2, att_edge2, b2):
    global LAST_EXEC_NS
    LAST_EXEC_NS = []
    trace = os.environ.get("BASSGNN_TRACE", "0") == "1"

    x = np.asarray(x, np.float32)
    ew = np.asarray(edge_weight, np.float32).reshape(-1)
    plan = _plan(np.asarray(edge_index))
    core_ids = list(range(NCORES))

    k1 = np.array([W_edge1[0, h * 64:(h + 1) * 64] @ att_edge1[h]
                   for h in range(2)], np.float32)
    k2 = np.array([W_edge2[0, :64] @ att_edge2[0]], np.float32)
    W1c = _wcat(np.asarray(W1, np.float32), np.asarray(att_src1),
                np.asarray(att_dst1), 2, 64)
    W2c = _wcat(np.asarray(W2, np.float32), np.asarray(att_src2),
                np.asarray(att_dst2), 1, 64)

    # ---- P1 ----
    order = plan["order"]
    xT = np.ascontiguousarray(x.T).astype(NPBF)
    nc1 = _build_proj(132, 128)
    maps1 = []
    for c in range(NCORES):
        nodes = order[c::NCORES]
        xTc = np.zeros((P, NROWS), NPBF)
        xTc[:, :nodes.size] = xT[:, nodes]
        maps1.append({"xT": xTc, "wcat": W1c.astype(NPBF)})
    r1 = run_bass_kernel_spmd(nc1, maps1, core_ids, trace=trace)
    if trace:
        LAST_EXEC_NS.append(r1.exec_time_ns)
    proj1 = _collect(plan, r1.results, "hb")             # [N, 132] f32
    h1b = proj1[:, :128].astype(NPBF)
    att1 = proj1[:, 128:].astype(np.float32)

    # ---- P2 ----
    nc2 = _build_agg(plan["KT"], 2, 64, relu=True, proj_cols=66)
    maps2 = _gather_inputs(plan, h1b, att1, ew, k1, np.asarray(b1), 2, 64,
                           w2c=W2c.astype(NPBF))
    r2 = run_bass_kernel_spmd(nc2, maps2, core_ids, trace=trace)
    if trace:
        LAST_EXEC_NS.append(r2.exec_time_ns)
    h2 = _collect(plan, r2.results, "out")
    h2b = h2[:, :64].astype(NPBF)
    att2 = h2[:, 64:66].astype(np.float32)

    # ---- P3 ----
    nc3 = _build_agg(plan["KT"], 1, 64, relu=False, proj_cols=0)
    maps3 = _gather_inputs(plan, h2b, att2, ew, k2, np.asarray(b2), 1, 64)
    r3 = run_bass_kernel_spmd(nc3, maps3, core_ids, trace=trace)
    if trace:
        LAST_EXEC_NS.append(r3.exec_time_ns)
    return _collect(plan, r3.results, "out").astype(np.float32)


# revision 20
# speedup vs baseline: 1.0452x; 1.0452x over previous
"""Two-layer GATConv (PyG-style, edge_dim=1, add_self_loops fill='mean') on
8 trn2 NeuronCores.

Strategy (v3: project-once, gather-h, contiguous-inner layouts)
---------------------------------------------------------------
Destinations are partitioned across the 8 cores (degree-sorted, dealt
round-robin).  Three device programs per kernel call:

  P1  per-node projection h1 = x @ [W1 | W1@A_src1 | W1@A_dst1]
      (bf16 matmul, one persistent xT load, grouped output DMAs).
  P2  layer-1 edge aggregation over a [128 dst x K_t slot] grid whose
      slot payloads are HOST-GATHERED h1 rows (bf16, hc-major k-inner
      blocks).  alpha on Pool, leaky+exp(+Z via accum_out) on ACT,
      normalized-attention product on DVE (all-bf16), bf16 fold-halving
      + f32 segment-reduce on DVE, bias on Pool, relu on ACT, fused
      layer-2 projection on PE.  Outputs h2 rows.
  P3  layer-2 edge aggregation, same grid, slots gathered from h2.

All FLOPs run on device; the host only moves data (gather/scatter of
rows, dtype rounding).
"""
import copy
import os

import numpy as np
import ml_dtypes

import concourse.bass as bass
import concourse.mybir as mybir
import concourse.tile as tile
from contextlib import ExitStack
from concourse.bass_utils import run_bass_kernel_spmd

NCORES = 8
P = 128
N = 50000
E = 800000
NPC = N // NCORES            # 6250 dsts per core
T = (NPC + P - 1) // P       # 49 tiles
NROWS = T * P                # 6272 rows per core (incl pad dsts)
G = 7                        # tiles per output-DMA group (49 = 7*7)
NEG_SLOPE = 0.2

F32 = mybir.dt.float32
BF16 = mybir.dt.bfloat16
NPBF = ml_dtypes.bfloat16

LAST_EXEC_NS = []


# --------------------------------------------------------------------------
# walrus workaround: cap sync waits per instruction (see v1 notes)
# --------------------------------------------------------------------------
def _split_waits(nc, limit=1):
    sem = nc.alloc_semaphore("wsplit_tmpl_sem")
    tmpl = {}
    for eng_ty, eng in nc.engines.items():
        tmpl[eng_ty] = eng.wait_ge(sem, 0).ins
    tmpl_names = {mi.name for mi in tmpl.values()}
    for f in nc.m.functions:
        for bb in f.blocks:
            insts = [i for i in bb.instructions if i.name not in tmpl_names]
            out = []
            for inst in insts:
                si = inst.sync_info
                waits = list(si.on_wait) if si and si.on_wait else []
                tn = type(inst).__name__
                eff = 0 if (tn == "InstDrain" or "Branch" in tn) else limit
                if len(waits) > eff:
                    head = waits[:-eff] if eff else waits
                    for w in head:
                        c = copy.deepcopy(tmpl[inst.engine])
                        c.name = f"I-wsplit-{nc.next_id()}"
                        c.sync_info = mybir.SyncInfo(on_wait=[w], on_update=[])
                        out.append(c)
                    inst.sync_info = mybir.SyncInfo(
                        on_wait=waits[-eff:] if eff else [],
                        on_update=list(si.on_update) if si.on_update else [],
                    )
                out.append(inst)
            bb.instructions = out


def _ap(root, extra_off, dims):
    return bass.AP(root.tensor, root.offset + extra_off, [list(d) for d in dims])


# --------------------------------------------------------------------------
# P1: per-node projection  h = x @ Wcat   (Wcat = [W | Wa_src | Wa_dst])
# --------------------------------------------------------------------------
def _build_proj(COLS, HC):
    nc = bass.Bass()
    xT = nc.dram_tensor("xT", [P, NROWS], BF16, kind="ExternalInput")
    wcat = nc.dram_tensor("wcat", [P, COLS], BF16, kind="ExternalInput")
    hb = nc.dram_tensor("hb", [NROWS, COLS], F32, kind="ExternalOutput")

    with ExitStack() as ctx:
        tc = ctx.enter_context(tile.TileContext(nc))
        pers = ctx.enter_context(tc.tile_pool(name="pers", bufs=1))
        sb = ctx.enter_context(tc.tile_pool(name="sb", bufs=2))
        ps = ctx.enter_context(tc.tile_pool(name="ps", bufs=4, space="PSUM"))

        wc = pers.tile([P, COLS], BF16)
        nc.sync.dma_start(out=wc[:], in_=wcat[:, :])
        xa = pers.tile([P, NROWS], BF16)
        nc.sync.dma_start(out=xa[:], in_=xT[:, :])

        hb_root = hb[:, :]
        for g in range(T // G):
            HG = sb.tile([P, G * COLS], F32, tag="HG")
            for j in range(G):
                t = g * G + j
                pg = ps.tile([P, COLS], F32, tag="pg")
                nc.tensor.matmul(out=pg[:], lhsT=xa[:, t * P:(t + 1) * P],
                                 rhs=wc[:], start=True, stop=True)
                nc.scalar.copy(out=HG[:, j * COLS:(j + 1) * COLS], in_=pg[:])
            hb_ap = _ap(hb_root, g * G * P * COLS,
                        [(COLS, P), (P * COLS, G), (1, COLS)])
            nc.sync.dma_start(out=hb_ap, in_=HG[:])

    _split_waits(nc)
    return nc


# --------------------------------------------------------------------------
# P2/P3: edge aggregation over the slot grid (slot payload = gathered h)
# --------------------------------------------------------------------------
def _build_agg(KT, H, C, relu, proj_cols):
    """hs blocks per tile: [HC, K_t] (hc-major, k contiguous).
    asr: [P, H*SK] (h-major).  ads: [P, H*T]."""
    HC = H * C
    SK = sum(KT)
    OUTC = proj_cols if proj_cols else HC

    nc = bass.Bass()
    hs = nc.dram_tensor("hs", [P, SK * HC], BF16, kind="ExternalInput")
    asr = nc.dram_tensor("asr", [P, H * SK], F32, kind="ExternalInput")
    ads = nc.dram_tensor("ads", [P, H * T], F32, kind="ExternalInput")
    warr = nc.dram_tensor("warr", [P, SK], F32, kind="ExternalInput")
    invc = nc.dram_tensor("invc", [P, T], F32, kind="ExternalInput")
    kk = nc.dram_tensor("kk", [P, H], F32, kind="ExternalInput")
    bvec = nc.dram_tensor("bvec", [P, HC], F32, kind="ExternalInput")
    if proj_cols:
        w2c = nc.dram_tensor("w2c", [P, proj_cols], BF16, kind="ExternalInput")
        idt = nc.dram_tensor("idt", [P, P], BF16, kind="ExternalInput")
    outp = nc.dram_tensor("out", [NROWS, OUTC], F32, kind="ExternalOutput")

    with ExitStack() as ctx:
        tc = ctx.enter_context(tile.TileContext(nc))
        pers = ctx.enter_context(tc.tile_pool(name="pers", bufs=1))
        hp = ctx.enter_context(tc.tile_pool(name="hp", bufs=3))
        sb = ctx.enter_context(tc.tile_pool(name="sb", bufs=3))
        og = ctx.enter_context(tc.tile_pool(name="og", bufs=2))
        if proj_cols:
            ps = ctx.enter_context(tc.tile_pool(name="ps", bufs=3, space="PSUM"))

        asr_t = pers.tile([P, H * SK], F32)
        nc.sync.dma_start(out=asr_t[:], in_=asr[:, :])
        ads_t = pers.tile([P, H * T], F32)
        nc.sync.dma_start(out=ads_t[:], in_=ads[:, :])
        wall = pers.tile([P, SK], F32)
        nc.sync.dma_start(out=wall[:], in_=warr[:, :])
        iva = pers.tile([P, T], F32)
        nc.sync.dma_start(out=iva[:], in_=invc[:, :])
        kt = pers.tile([P, H], F32)
        nc.sync.dma_start(out=kt[:], in_=kk[:, :])
        bt = pers.tile([P, HC], F32)
        nc.sync.dma_start(out=bt[:], in_=bvec[:, :])
        if proj_cols:
            w2t = pers.tile([P, proj_cols], BF16)
            nc.sync.dma_start(out=w2t[:], in_=w2c[:, :])
            idtt = pers.tile([P, P], BF16)
            nc.sync.dma_start(out=idtt[:], in_=idt[:, :])

        kpitch = kt[:].ap[0][0]
        aspitch = asr_t[:].ap[0][0]
        adpitch = ads_t[:].ap[0][0]
        out_root = outp[:, :]
        cb = 0
        OG = None
        for t in range(T):
            K = KT[t]
            j = t % G
            if j == 0:
                OG = og.tile([P, G * OUTC], F32, tag="OG")
            HS = hp.tile([P, K * HC], BF16, tag="HS")
            nc.sync.dma_start(out=HS[:], in_=hs[:, cb * HC:(cb + K) * HC])

            # alpha[p, h, k] = a_src[slot] + a_dst[dst] + w*k_h   (Pool)
            A = sb.tile([P, H * K], F32, tag="A")
            a0 = A[:]
            apitch = a0.ap[0][0]
            A3 = _ap(a0, 0, [(apitch, P), (K, H), (1, K)])
            asrc_b = _ap(asr_t[:], cb, [(aspitch, P), (SK, H), (1, K)])
            adst_b = _ap(ads_t[:], t, [(adpitch, P), (T, H), (0, K)])
            nc.gpsimd.tensor_tensor(out=A3, in0=asrc_b, in1=adst_b,
                                    op=mybir.AluOpType.add)
            wt0 = wall[:, cb:cb + K]
            wpitch = wt0.ap[0][0]
            WK = sb.tile([P, H * K], F32, tag="WK")
            WK3 = _ap(WK[:], 0, [(WK[:].ap[0][0], P), (K, H), (1, K)])
            w_b = _ap(wt0, 0, [(wpitch, P), (0, H), (1, K)])
            kk_b = _ap(kt[:], 0, [(kpitch, P), (1, H), (0, K)])
            nc.gpsimd.tensor_tensor(out=WK3, in0=w_b, in1=kk_b,
                                    op=mybir.AluOpType.mult)
            nc.gpsimd.tensor_tensor(out=A3, in0=A3, in1=WK3,
                                    op=mybir.AluOpType.add)
            # self-loop alpha correction at k = K-1
            LA = sb.tile([P, 1], F32, tag="LA")
            nc.vector.tensor_reduce(out=LA[:], in_=wt0,
                                    axis=mybir.AxisListType.X,
                                    op=mybir.AluOpType.add)
            nc.gpsimd.tensor_tensor(out=LA[:], in0=LA[:], in1=iva[:, t:t + 1],
                                    op=mybir.AluOpType.mult)
            A_self = _ap(a0, K - 1, [(apitch, P), (K, H)])
            kk_b2 = _ap(kt[:], 0, [(kpitch, P), (1, H)])
            nc.vector.scalar_tensor_tensor(out=A_self, in0=kk_b2,
                                           scalar=LA[:], in1=A_self,
                                           op0=mybir.AluOpType.mult,
                                           op1=mybir.AluOpType.add)
            # leaky relu (DVE STT) + exp (ACT); Z via accum_out
            AL = sb.tile([P, H * K], F32, tag="AL")
            nc.vector.scalar_tensor_tensor(out=AL[:], in0=A[:],
                                           scalar=NEG_SLOPE, in1=A[:],
                                           op0=mybir.AluOpType.mult,
                                           op1=mybir.AluOpType.max)
            PP = sb.tile([P, H * K], BF16, tag="PP")
            Z = sb.tile([P, H], F32, tag="Z")
            for h in range(H):
                nc.scalar.activation(out=PP[:, h * K:(h + 1) * K],
                                     in_=AL[:, h * K:(h + 1) * K],
                                     func=mybir.ActivationFunctionType.Exp,
                                     accum_out=Z[:, h:h + 1])
            Zr = sb.tile([P, H], F32, tag="Zr")
            nc.vector.reciprocal(out=Zr[:], in_=Z[:])
            PPn = sb.tile([P, H * K], BF16, tag="PPn")
            pn0 = PPn[:]
            pnpitch = pn0.ap[0][0]
            Zr_b = _ap(Zr[:], 0, [(Zr[:].ap[0][0], P), (1, H), (0, K)])
            PP3 = _ap(PP[:], 0, [(PP[:].ap[0][0], P), (K, H), (1, K)])
            PPn3 = _ap(pn0, 0, [(pnpitch, P), (K, H), (1, K)])
            nc.vector.tensor_tensor(out=PPn3, in0=PP3, in1=Zr_b,
                                    op=mybir.AluOpType.mult)
            # PROD[p, hc, k] = HS[p, hc, k] * PPn[p, h, k]  (hc-major blocks;
            # one 2-dim-AP instr per head -> DVE 2x mode engages)
            PROD = hp.tile([P, HC * K], BF16, tag="PROD")
            p0 = PROD[:]
            ppitch = p0.ap[0][0]
            h0 = HS[:]
            hpitch = h0.ap[0][0]
            for h in range(H):
                nc.vector.tensor_tensor(
                    out=_ap(p0, h * C * K, [(ppitch, P), (K, C), (1, K)]),
                    in0=_ap(h0, h * C * K, [(hpitch, P), (K, C), (1, K)]),
                    in1=_ap(pn0, h * K, [(pnpitch, P), (0, C), (1, K)]),
                    op=mybir.AluOpType.mult)
            # one f32 fold (pairs k, k+K/2) split Pool/DVE by hc rows,
            # then contiguous-inner f32 reduce (DVE)
            nf = K // 2
            F1 = hp.tile([P, HC * nf], F32, tag="F1")
            f0 = F1[:]
            fpitch = f0.ap[0][0]
            SPLIT = 96 if H == 2 else 56    # hc rows handled by Pool
            for eng, lo, hi in ((nc.gpsimd, 0, SPLIT), (nc.vector, SPLIT, HC)):
                dstap = _ap(f0, lo * nf, [(fpitch, P), (nf, hi - lo), (1, nf)])
                s0ap = _ap(p0, lo * K, [(ppitch, P), (K, hi - lo), (1, nf)])
                s1ap = _ap(p0, lo * K + nf, [(ppitch, P), (K, hi - lo), (1, nf)])
                eng.tensor_tensor(out=dstap, in0=s0ap, in1=s1ap,
                                  op=mybir.AluOpType.add)
            O = sb.tile([P, HC], F32, tag="O")
            Or = _ap(O[:], 0, [(O[:].ap[0][0], P), (1, HC)])
            F1r = _ap(f0, 0, [(fpitch, P), (nf, HC), (1, nf)])
            nc.vector.tensor_reduce(out=Or, in_=F1r,
                                    axis=mybir.AxisListType.X,
                                    op=mybir.AluOpType.add)
            # + bias (Pool); b is zero in this workload but kept general
            if proj_cols:
                Ob = sb.tile([P, HC], F32, tag="Ob")
                nc.gpsimd.tensor_tensor(out=Ob[:], in0=O[:], in1=bt[:],
                                        op=mybir.AluOpType.add)
                R = sb.tile([P, HC], BF16, tag="R")
                nc.scalar.activation(out=R[:], in_=Ob[:],
                                     func=mybir.ActivationFunctionType.Relu)
                tp = ps.tile([P, P], BF16, tag="tp")
                nc.tensor.transpose(out=tp[:], in_=R[:], identity=idtt[:])
                rt = sb.tile([P, P], BF16, tag="rt")
                nc.scalar.copy(out=rt[:], in_=tp[:])
                h2p = ps.tile([P, proj_cols], F32, tag="h2p")
                nc.tensor.matmul(out=h2p[:], lhsT=rt[:], rhs=w2t[:],
                                 start=True, stop=True)
                nc.scalar.copy(out=OG[:, j * OUTC:(j + 1) * OUTC], in_=h2p[:])
            else:
                nc.gpsimd.tensor_tensor(out=OG[:, j * OUTC:(j + 1) * OUTC],
                                        in0=O[:], in1=bt[:],
                                        op=mybir.AluOpType.add)
            if j == G - 1:
                g0i = t - G + 1
                out_ap = _ap(out_root, g0i * P * OUTC,
                             [(OUTC, P), (P * OUTC, G), (1, OUTC)])
                nc.sync.dma_start(out=out_ap, in_=OG[:])
            cb += K

    _split_waits(nc)
    return nc


# --------------------------------------------------------------------------
# host-side planning (identical partition to baseline)
# --------------------------------------------------------------------------
def _plan(edge_index):
    src = np.asarray(edge_index[0], dtype=np.int64)
    dst = np.asarray(edge_index[1], dtype=np.int64)
    deg = np.bincount(dst, minlength=N)
    order = np.argsort(-deg, kind="stable")
    rank_of = np.empty(N, np.int64)
    rank_of[order] = np.arange(N)
    core_of = (rank_of % NCORES).astype(np.int64)
    loc_of = (rank_of // NCORES).astype(np.int64)

    KT = []
    for t in range(T):
        r0 = min(1024 * t, N - 1)
        k = int(deg[order[r0]]) + 1
        KT.append(k + (k & 1))            # even K -> clean fold pairing
    KT = [max(k, 2) for k in KT]
    cbs = np.concatenate([[0], np.cumsum(KT)])

    eorder = np.argsort(dst, kind="stable")
    starts = np.concatenate([[0], np.cumsum(deg)])
    kpos_sorted = np.arange(E) - starts[dst[eorder]]
    kpos = np.empty(E, np.int64)
    kpos[eorder] = kpos_sorted

    e_core = core_of[dst]
    e_loc = loc_of[dst]
    e_t = e_loc >> 7
    e_p = e_loc & 127
    e_scol = cbs[e_t] + kpos

    return dict(src=src, dst=dst, deg=deg, order=order, core_of=core_of,
                loc_of=loc_of, KT=KT, cbs=cbs, e_core=e_core, e_t=e_t,
                e_p=e_p, e_scol=e_scol, kpos=kpos)


def _gather_inputs(plan, hb_full, att_full, ew, kvec, bias, H, C, w2c=None):
    """Per-core input maps for one aggregation layer.
    hb_full: [N, H*C] bf16; att_full: [N, 2H] f32 ([a_src | a_dst])."""
    HC = H * C
    KT, cbs = plan["KT"], plan["cbs"]
    SK = int(cbs[-1])
    src, e_core = plan["src"], plan["e_core"]
    e_p, e_t, kpos = plan["e_p"], plan["e_t"], plan["kpos"]
    e_scol = plan["e_scol"]
    order, deg = plan["order"], plan["deg"]
    KTa = np.array(KT)

    maps = []
    for c in range(NCORES):
        m = e_core == c
        asr = np.full((P, H, SK), -1e4, np.float32)
        war = np.zeros((P, SK), np.float32)
        asr[e_p[m], :, e_scol[m]] = att_full[src[m], :H]
        war[e_p[m], e_scol[m]] = ew[m]
        nodes = order[c::NCORES]
        loc = np.arange(nodes.size)
        tt = loc >> 7
        pp = loc & 127
        self_col = cbs[tt] + KTa[tt] - 1
        asr[pp, :, self_col] = att_full[nodes, :H]
        ads = np.zeros((P, H, T), np.float32)
        ads[pp, :, tt] = att_full[nodes, H:]
        iv = np.ones((P, T), np.float32)
        iv[pp, tt] = 1.0 / np.maximum(deg[nodes], 1.0)
        # hs: per-tile [P, HC, K_t] blocks (hc-major, k inner); built
        # k-major vectorized then transposed per tile
        hsr = np.zeros((P, SK, HC), NPBF)
        hsr[e_p[m], e_scol[m]] = hb_full[src[m]]
        hsr[pp, self_col] = hb_full[nodes]
        hs2 = np.empty((P, SK * HC), NPBF)
        for t in range(T):
            cb, K = int(cbs[t]), int(KTa[t])
            hs2[:, cb * HC:(cb + K) * HC] = np.ascontiguousarray(
                hsr[:, cb:cb + K, :].transpose(0, 2, 1)).reshape(P, K * HC)
        mp = {
            "hs": hs2,
            "asr": np.ascontiguousarray(asr.reshape(P, H * SK)),
            "ads": np.ascontiguousarray(ads.reshape(P, H * T)),
            "warr": war,
            "invc": iv,
            "kk": np.tile(kvec.reshape(1, H).astype(np.float32), (P, 1)),
            "bvec": np.tile(bias.reshape(1, -1).astype(np.float32), (P, 1)),
        }
        if w2c is not None:
            mp["w2c"] = w2c
            mp["idt"] = np.eye(P, dtype=NPBF)
        maps.append(mp)
    return maps


def _collect(plan, results, key):
    stack = np.stack([np.asarray(r[key]) for r in results])
    return stack[plan["core_of"], plan["loc_of"], :]


def _wcat(W, att_src, att_dst, H, C):
    Wa_s = np.stack([W[:, h * C:(h + 1) * C] @ att_src[h] for h in range(H)], 1)
    Wa_d = np.stack([W[:, h * C:(h + 1) * C] @ att_dst[h] for h in range(H)], 1)
    return np.concatenate([W, Wa_s, Wa_d], axis=1).astype(np.float32)


def kernel(x, edge_index, edge_weight, W1, att_src1, att_dst1, W_edge1,
           att_edge1, b1, W2, att_src2, att_dst2, W_edge2, att_edge2, b2):
    global LAST_EXEC_NS
    LAST_EXEC_NS = []
    trace = os.environ.get("BASSGNN_TRACE", "0") == "1"

    x = np.asarray(x, np.float32)
    ew = np.asarray(edge_weight, np.float32).reshape(-1)
    plan = _plan(np.asarray(edge_index))
    core_ids = list(range(NCORES))

    k1 = np.array([W_edge1[0, h * 64:(h + 1) * 64] @ att_edge1[h]
                   for h in range(2)], np.float32)
    k2 = np.array([W_edge2[0, :64] @ att_edge2[0]], np.float32)
    W1c = _wcat(np.asarray(W1, np.float32), np.asarray(att_src1),
                np.asarray(att_dst1), 2, 64)
    W2c = _wcat(np.asarray(W2, np.float32), np.asarray(att_src2),
                np.asarray(att_dst2), 1, 64)

    # ---- P1 ----
    order = plan["order"]
    xT = np.ascontiguousarray(x.T).astype(NPBF)
    nc1 = _build_proj(132, 128)
    maps1 = []
    for c in range(NCORES):
        nodes = order[c::NCORES]
        xTc = np.zeros((P, NROWS), NPBF)
        xTc[:, :nodes.size] = xT[:, nodes]
        maps1.append({"xT": xTc, "wcat": W1c.astype(NPBF)})
    r1 = run_bass_kernel_spmd(nc1, maps1, core_ids, trace=trace)
    if trace:
        LAST_EXEC_NS.append(r1.exec_time_ns)
    proj1 = _collect(plan, r1.results, "hb")             # [N, 132] f32
    h1b = proj1[:, :128].astype(NPBF)
    att1 = proj1[:, 128:].astype(np.float32)

    # ---- P2 ----
    nc2 = _build_agg(plan["KT"], 2, 64, relu=True, proj_cols=66)
    maps2 = _gather_inputs(plan, h1b, att1, ew, k1, np.asarray(b1), 2, 64,
                           w2c=W2c.astype(NPBF))
    r2 = run_bass_kernel_spmd(nc2, maps2, core_ids, trace=trace)
    if trace:
        LAST_EXEC_NS.append(r2.exec_time_ns)
    h2 = _collect(plan, r2.results, "out")
    h2b = h2[:, :64].astype(NPBF)
    att2 = h2[:, 64:66].astype(np.float32)

    # ---- P3 ----
    nc3 = _build_agg(plan["KT"], 1, 64, relu=False, proj_cols=0)
    maps3 = _gather_inputs(plan, h2b, att2, ew, k2, np.asarray(b2), 1, 64)
    r3 = run_bass_kernel_spmd(nc3, maps3, core_ids, trace=trace)
    if trace:
        LAST_EXEC_NS.append(r3.exec_time_ns)
    return _collect(plan, r3.results, "out").astype(np.float32)


# revision 23
# speedup vs baseline: 1.2348x; 1.1815x over previous
"""Two-layer GATConv (PyG-style, edge_dim=1, add_self_loops fill='mean') on
8 trn2 NeuronCores.

Strategy (v3: project-once, gather-h, contiguous-inner layouts)
---------------------------------------------------------------
Destinations are partitioned across the 8 cores (degree-sorted, dealt
round-robin).  Three device programs per kernel call:

  P1  per-node projection h1 = x @ [W1 | W1@A_src1 | W1@A_dst1]
      (bf16 matmul, one persistent xT load, grouped output DMAs).
  P2  layer-1 edge aggregation over a [128 dst x K_t slot] grid whose
      slot payloads are HOST-GATHERED h1 rows (bf16, hc-major k-inner
      blocks).  alpha on Pool, leaky+exp(+Z via accum_out) on ACT,
      normalized-attention product on DVE (all-bf16), bf16 fold-halving
      + f32 segment-reduce on DVE, bias on Pool, relu on ACT, fused
      layer-2 projection on PE.  Outputs h2 rows.
  P3  layer-2 edge aggregation, same grid, slots gathered from h2.

All FLOPs run on device; the host only moves data (gather/scatter of
rows, dtype rounding).
"""
import copy
import os

import numpy as np
import ml_dtypes

import concourse.bass as bass
import concourse.mybir as mybir
import concourse.tile as tile
from contextlib import ExitStack
from concourse.bass_utils import run_bass_kernel_spmd

NCORES = 8
P = 128
N = 50000
E = 800000
NPC = N // NCORES            # 6250 dsts per core
T = (NPC + P - 1) // P       # 49 tiles
NROWS = T * P                # 6272 rows per core (incl pad dsts)
G = 7                        # tiles per output-DMA group (49 = 7*7)
NEG_SLOPE = 0.2

F32 = mybir.dt.float32
BF16 = mybir.dt.bfloat16
NPBF = ml_dtypes.bfloat16

LAST_EXEC_NS = []


# --------------------------------------------------------------------------
# walrus workaround: cap sync waits per instruction (see v1 notes)
# --------------------------------------------------------------------------
def _split_waits(nc, limit=1):
    sem = nc.alloc_semaphore("wsplit_tmpl_sem")
    tmpl = {}
    for eng_ty, eng in nc.engines.items():
        tmpl[eng_ty] = eng.wait_ge(sem, 0).ins
    tmpl_names = {mi.name for mi in tmpl.values()}
    for f in nc.m.functions:
        for bb in f.blocks:
            insts = [i for i in bb.instructions if i.name not in tmpl_names]
            out = []
            for inst in insts:
                si = inst.sync_info
                waits = list(si.on_wait) if si and si.on_wait else []
                tn = type(inst).__name__
                eff = 0 if (tn == "InstDrain" or "Branch" in tn) else limit
                if len(waits) > eff:
                    head = waits[:-eff] if eff else waits
                    for w in head:
                        c = copy.deepcopy(tmpl[inst.engine])
                        c.name = f"I-wsplit-{nc.next_id()}"
                        c.sync_info = mybir.SyncInfo(on_wait=[w], on_update=[])
                        out.append(c)
                    inst.sync_info = mybir.SyncInfo(
                        on_wait=waits[-eff:] if eff else [],
                        on_update=list(si.on_update) if si.on_update else [],
                    )
                out.append(inst)
            bb.instructions = out


def _ap(root, extra_off, dims):
    return bass.AP(root.tensor, root.offset + extra_off, [list(d) for d in dims])


# --------------------------------------------------------------------------
# P1: per-node projection  h = x @ Wcat   (Wcat = [W | Wa_src | Wa_dst])
# --------------------------------------------------------------------------
def _build_proj(COLS, HC):
    nc = bass.Bass()
    xT = nc.dram_tensor("xT", [P, NROWS], BF16, kind="ExternalInput")
    wcat = nc.dram_tensor("wcat", [P, COLS], BF16, kind="ExternalInput")
    hb = nc.dram_tensor("hb", [NROWS, COLS], F32, kind="ExternalOutput")

    with ExitStack() as ctx:
        tc = ctx.enter_context(tile.TileContext(nc))
        pers = ctx.enter_context(tc.tile_pool(name="pers", bufs=1))
        sb = ctx.enter_context(tc.tile_pool(name="sb", bufs=2))
        ps = ctx.enter_context(tc.tile_pool(name="ps", bufs=4, space="PSUM"))

        wc = pers.tile([P, COLS], BF16)
        nc.sync.dma_start(out=wc[:], in_=wcat[:, :])
        xa = pers.tile([P, NROWS], BF16)
        nc.sync.dma_start(out=xa[:], in_=xT[:, :])

        hb_root = hb[:, :]
        for g in range(T // G):
            HG = sb.tile([P, G * COLS], F32, tag="HG")
            for j in range(G):
                t = g * G + j
                pg = ps.tile([P, COLS], F32, tag="pg")
                nc.tensor.matmul(out=pg[:], lhsT=xa[:, t * P:(t + 1) * P],
                                 rhs=wc[:], start=True, stop=True)
                if t % 2 == 0:
                    nc.scalar.copy(out=HG[:, j * COLS:(j + 1) * COLS], in_=pg[:])
                else:
                    nc.vector.tensor_scalar_add(
                        out=HG[:, j * COLS:(j + 1) * COLS], in0=pg[:],
                        scalar1=0.0)
            hb_ap = _ap(hb_root, g * G * P * COLS,
                        [(COLS, P), (P * COLS, G), (1, COLS)])
            nc.sync.dma_start(out=hb_ap, in_=HG[:])

    _split_waits(nc)
    return nc


# --------------------------------------------------------------------------
# P2/P3: edge aggregation over the slot grid (slot payload = gathered h)
# --------------------------------------------------------------------------
def _build_agg(KT, H, C, relu, proj_cols):
    """hs blocks per tile: [HC, K_t] (hc-major, k contiguous).
    asr: [P, H*SK] (h-major).  ads: [P, H*T]."""
    HC = H * C
    SK = sum(KT)
    OUTC = proj_cols if proj_cols else HC

    nc = bass.Bass()
    hs = nc.dram_tensor("hs", [P, SK * HC], BF16, kind="ExternalInput")
    asr = nc.dram_tensor("asr", [P, H * SK], F32, kind="ExternalInput")
    ads = nc.dram_tensor("ads", [P, H * T], F32, kind="ExternalInput")
    warr = nc.dram_tensor("warr", [P, SK], F32, kind="ExternalInput")
    invc = nc.dram_tensor("invc", [P, T], F32, kind="ExternalInput")
    kk = nc.dram_tensor("kk", [P, H], F32, kind="ExternalInput")
    bvec = nc.dram_tensor("bvec", [P, HC], F32, kind="ExternalInput")
    if proj_cols:
        w2c = nc.dram_tensor("w2c", [P, proj_cols], BF16, kind="ExternalInput")
        idt = nc.dram_tensor("idt", [P, P], BF16, kind="ExternalInput")
    outp = nc.dram_tensor("out", [NROWS, OUTC], F32, kind="ExternalOutput")

    with ExitStack() as ctx:
        tc = ctx.enter_context(tile.TileContext(nc))
        pers = ctx.enter_context(tc.tile_pool(name="pers", bufs=1))
        hp = ctx.enter_context(tc.tile_pool(name="hp", bufs=3))
        sb = ctx.enter_context(tc.tile_pool(name="sb", bufs=3))
        og = ctx.enter_context(tc.tile_pool(name="og", bufs=2))
        if proj_cols:
            ps = ctx.enter_context(tc.tile_pool(name="ps", bufs=3, space="PSUM"))

        asr_t = pers.tile([P, H * SK], F32)
        nc.sync.dma_start(out=asr_t[:], in_=asr[:, :])
        ads_t = pers.tile([P, H * T], F32)
        nc.sync.dma_start(out=ads_t[:], in_=ads[:, :])
        wall = pers.tile([P, SK], F32)
        nc.sync.dma_start(out=wall[:], in_=warr[:, :])
        iva = pers.tile([P, T], F32)
        nc.sync.dma_start(out=iva[:], in_=invc[:, :])
        kt = pers.tile([P, H], F32)
        nc.sync.dma_start(out=kt[:], in_=kk[:, :])
        bt = pers.tile([P, HC], F32)
        nc.sync.dma_start(out=bt[:], in_=bvec[:, :])
        if proj_cols:
            w2t = pers.tile([P, proj_cols], BF16)
            nc.sync.dma_start(out=w2t[:], in_=w2c[:, :])
            idtt = pers.tile([P, P], BF16)
            nc.sync.dma_start(out=idtt[:], in_=idt[:, :])

        kpitch = kt[:].ap[0][0]
        aspitch = asr_t[:].ap[0][0]
        adpitch = ads_t[:].ap[0][0]
        out_root = outp[:, :]
        cb = 0
        OG = None
        for t in range(T):
            K = KT[t]
            j = t % G
            if j == 0:
                OG = og.tile([P, G * OUTC], F32, tag="OG")
            HS = hp.tile([P, K * HC], BF16, tag="HS")
            nc.sync.dma_start(out=HS[:], in_=hs[:, cb * HC:(cb + K) * HC])

            # alpha[p, h, k] = a_src[slot] + a_dst[dst] + w*k_h   (Pool)
            A = sb.tile([P, H * K], F32, tag="A")
            a0 = A[:]
            apitch = a0.ap[0][0]
            A3 = _ap(a0, 0, [(apitch, P), (K, H), (1, K)])
            asrc_b = _ap(asr_t[:], cb, [(aspitch, P), (SK, H), (1, K)])
            adst_b = _ap(ads_t[:], t, [(adpitch, P), (T, H), (0, K)])
            nc.gpsimd.tensor_tensor(out=A3, in0=asrc_b, in1=adst_b,
                                    op=mybir.AluOpType.add)
            wt0 = wall[:, cb:cb + K]
            wpitch = wt0.ap[0][0]
            WK = sb.tile([P, H * K], F32, tag="WK")
            WK3 = _ap(WK[:], 0, [(WK[:].ap[0][0], P), (K, H), (1, K)])
            w_b = _ap(wt0, 0, [(wpitch, P), (0, H), (1, K)])
            kk_b = _ap(kt[:], 0, [(kpitch, P), (1, H), (0, K)])
            nc.gpsimd.tensor_tensor(out=WK3, in0=w_b, in1=kk_b,
                                    op=mybir.AluOpType.mult)
            nc.gpsimd.tensor_tensor(out=A3, in0=A3, in1=WK3,
                                    op=mybir.AluOpType.add)
            # self-loop alpha correction at k = K-1
            LA = sb.tile([P, 1], F32, tag="LA")
            nc.vector.tensor_reduce(out=LA[:], in_=wt0,
                                    axis=mybir.AxisListType.X,
                                    op=mybir.AluOpType.add)
            nc.gpsimd.tensor_tensor(out=LA[:], in0=LA[:], in1=iva[:, t:t + 1],
                                    op=mybir.AluOpType.mult)
            A_self = _ap(a0, K - 1, [(apitch, P), (K, H)])
            kk_b2 = _ap(kt[:], 0, [(kpitch, P), (1, H)])
            nc.vector.scalar_tensor_tensor(out=A_self, in0=kk_b2,
                                           scalar=LA[:], in1=A_self,
                                           op0=mybir.AluOpType.mult,
                                           op1=mybir.AluOpType.add)
            # leaky relu (DVE STT) + exp (ACT); Z via accum_out
            AL = sb.tile([P, H * K], F32, tag="AL")
            nc.vector.scalar_tensor_tensor(out=AL[:], in0=A[:],
                                           scalar=NEG_SLOPE, in1=A[:],
                                           op0=mybir.AluOpType.mult,
                                           op1=mybir.AluOpType.max)
            PP = sb.tile([P, H * K], BF16, tag="PP")
            Z = sb.tile([P, H], F32, tag="Z")
            for h in range(H):
                nc.scalar.activation(out=PP[:, h * K:(h + 1) * K],
                                     in_=AL[:, h * K:(h + 1) * K],
                                     func=mybir.ActivationFunctionType.Exp,
                                     accum_out=Z[:, h:h + 1])
            Zr = sb.tile([P, H], F32, tag="Zr")
            nc.vector.reciprocal(out=Zr[:], in_=Z[:])
            PPn = sb.tile([P, H * K], BF16, tag="PPn")
            pn0 = PPn[:]
            pnpitch = pn0.ap[0][0]
            Zr_b = _ap(Zr[:], 0, [(Zr[:].ap[0][0], P), (1, H), (0, K)])
            PP3 = _ap(PP[:], 0, [(PP[:].ap[0][0], P), (K, H), (1, K)])
            PPn3 = _ap(pn0, 0, [(pnpitch, P), (K, H), (1, K)])
            nc.gpsimd.tensor_tensor(out=PPn3, in0=PP3, in1=Zr_b,
                                    op=mybir.AluOpType.mult)
            # PROD[p, hc, k] = HS[p, hc, k] * PPn[p, h, k]  (hc-major blocks;
            # one 2-dim-AP instr per head -> DVE 2x mode engages)
            PROD = hp.tile([P, HC * K], BF16, tag="PROD")
            p0 = PROD[:]
            ppitch = p0.ap[0][0]
            h0 = HS[:]
            hpitch = h0.ap[0][0]
            for h in range(H):
                nc.vector.tensor_tensor(
                    out=_ap(p0, h * C * K, [(ppitch, P), (K, C), (1, K)]),
                    in0=_ap(h0, h * C * K, [(hpitch, P), (K, C), (1, K)]),
                    in1=_ap(pn0, h * K, [(pnpitch, P), (0, C), (1, K)]),
                    op=mybir.AluOpType.mult)
            # one f32 fold (pairs k, k+K/2) split Pool/DVE by hc rows,
            # then contiguous-inner f32 reduce (DVE)
            nf = K // 2
            F1 = hp.tile([P, HC * nf], F32, tag="F1")
            f0 = F1[:]
            fpitch = f0.ap[0][0]
            SPLIT = 104 if H == 2 else 48   # hc rows handled by Pool
            for eng, lo, hi in ((nc.gpsimd, 0, SPLIT), (nc.vector, SPLIT, HC)):
                dstap = _ap(f0, lo * nf, [(fpitch, P), (nf, hi - lo), (1, nf)])
                s0ap = _ap(p0, lo * K, [(ppitch, P), (K, hi - lo), (1, nf)])
                s1ap = _ap(p0, lo * K + nf, [(ppitch, P), (K, hi - lo), (1, nf)])
                eng.tensor_tensor(out=dstap, in0=s0ap, in1=s1ap,
                                  op=mybir.AluOpType.add)
            O = sb.tile([P, HC], F32, tag="O")
            Or = _ap(O[:], 0, [(O[:].ap[0][0], P), (1, HC)])
            F1r = _ap(f0, 0, [(fpitch, P), (nf, HC), (1, nf)])
            nc.vector.tensor_reduce(out=Or, in_=F1r,
                                    axis=mybir.AxisListType.X,
                                    op=mybir.AluOpType.add)
            # + bias (Pool); b is zero in this workload but kept general
            if proj_cols:
                Ob = sb.tile([P, HC], F32, tag="Ob")
                nc.gpsimd.tensor_tensor(out=Ob[:], in0=O[:], in1=bt[:],
                                        op=mybir.AluOpType.add)
                R = sb.tile([P, HC], BF16, tag="R")
                nc.scalar.activation(out=R[:], in_=Ob[:],
                                     func=mybir.ActivationFunctionType.Relu)
                tp = ps.tile([P, P], BF16, tag="tp")
                nc.tensor.transpose(out=tp[:], in_=R[:], identity=idtt[:])
                rt = sb.tile([P, P], BF16, tag="rt")
                nc.scalar.copy(out=rt[:], in_=tp[:])
                h2p = ps.tile([P, proj_cols], F32, tag="h2p")
                nc.tensor.matmul(out=h2p[:], lhsT=rt[:], rhs=w2t[:],
                                 start=True, stop=True)
                nc.scalar.copy(out=OG[:, j * OUTC:(j + 1) * OUTC], in_=h2p[:])
            else:
                nc.gpsimd.tensor_tensor(out=OG[:, j * OUTC:(j + 1) * OUTC],
                                        in0=O[:], in1=bt[:],
                                        op=mybir.AluOpType.add)
            if j == G - 1:
                g0i = t - G + 1
                out_ap = _ap(out_root, g0i * P * OUTC,
                             [(OUTC, P), (P * OUTC, G), (1, OUTC)])
                nc.sync.dma_start(out=out_ap, in_=OG[:])
            cb += K

    _split_waits(nc)
    return nc


# --------------------------------------------------------------------------
# host-side planning (identical partition to baseline)
# --------------------------------------------------------------------------
def _plan(edge_index):
    src = np.asarray(edge_index[0], dtype=np.int64)
    dst = np.asarray(edge_index[1], dtype=np.int64)
    deg = np.bincount(dst, minlength=N)
    order = np.argsort(-deg, kind="stable")
    rank_of = np.empty(N, np.int64)
    rank_of[order] = np.arange(N)
    core_of = (rank_of % NCORES).astype(np.int64)
    loc_of = (rank_of // NCORES).astype(np.int64)

    KT = []
    for t in range(T):
        r0 = min(1024 * t, N - 1)
        k = int(deg[order[r0]]) + 1
        KT.append(k + (k & 1))            # even K -> clean fold pairing
    KT = [max(k, 2) for k in KT]
    cbs = np.concatenate([[0], np.cumsum(KT)])

    eorder = np.argsort(dst, kind="stable")
    starts = np.concatenate([[0], np.cumsum(deg)])
    kpos_sorted = np.arange(E) - starts[dst[eorder]]
    kpos = np.empty(E, np.int64)
    kpos[eorder] = kpos_sorted

    e_core = core_of[dst]
    e_loc = loc_of[dst]
    e_t = e_loc >> 7
    e_p = e_loc & 127
    e_scol = cbs[e_t] + kpos

    return dict(src=src, dst=dst, deg=deg, order=order, core_of=core_of,
                loc_of=loc_of, KT=KT, cbs=cbs, e_core=e_core, e_t=e_t,
                e_p=e_p, e_scol=e_scol, kpos=kpos)


def _gather_inputs(plan, hb_full, att_full, ew, kvec, bias, H, C, w2c=None):
    """Per-core input maps for one aggregation layer.
    hb_full: [N, H*C] bf16; att_full: [N, 2H] f32 ([a_src | a_dst])."""
    HC = H * C
    KT, cbs = plan["KT"], plan["cbs"]
    SK = int(cbs[-1])
    src, e_core = plan["src"], plan["e_core"]
    e_p, e_t, kpos = plan["e_p"], plan["e_t"], plan["kpos"]
    e_scol = plan["e_scol"]
    order, deg = plan["order"], plan["deg"]
    KTa = np.array(KT)

    maps = []
    for c in range(NCORES):
        m = e_core == c
        asr = np.full((P, H, SK), -1e4, np.float32)
        war = np.zeros((P, SK), np.float32)
        asr[e_p[m], :, e_scol[m]] = att_full[src[m], :H]
        war[e_p[m], e_scol[m]] = ew[m]
        nodes = order[c::NCORES]
        loc = np.arange(nodes.size)
        tt = loc >> 7
        pp = loc & 127
        self_col = cbs[tt] + KTa[tt] - 1
        asr[pp, :, self_col] = att_full[nodes, :H]
        ads = np.zeros((P, H, T), np.float32)
        ads[pp, :, tt] = att_full[nodes, H:]
        iv = np.ones((P, T), np.float32)
        iv[pp, tt] = 1.0 / np.maximum(deg[nodes], 1.0)
        # hs: per-tile [P, HC, K_t] blocks (hc-major, k inner); built
        # k-major vectorized then transposed per tile
        hsr = np.zeros((P, SK, HC), NPBF)
        hsr[e_p[m], e_scol[m]] = hb_full[src[m]]
        hsr[pp, self_col] = hb_full[nodes]
        hs2 = np.empty((P, SK * HC), NPBF)
        for t in range(T):
            cb, K = int(cbs[t]), int(KTa[t])
            hs2[:, cb * HC:(cb + K) * HC] = np.ascontiguousarray(
                hsr[:, cb:cb + K, :].transpose(0, 2, 1)).reshape(P, K * HC)
        mp = {
            "hs": hs2,
            "asr": np.ascontiguousarray(asr.reshape(P, H * SK)),
            "ads": np.ascontiguousarray(ads.reshape(P, H * T)),
            "warr": war,
            "invc": iv,
            "kk": np.tile(kvec.reshape(1, H).astype(np.float32), (P, 1)),
            "bvec": np.tile(bias.reshape(1, -1).astype(np.float32), (P, 1)),
        }
        if w2c is not None:
            mp["w2c"] = w2c
            mp["idt"] = np.eye(P, dtype=NPBF)
        maps.append(mp)
    return maps


def _collect(plan, results, key):
    stack = np.stack([np.asarray(r[key]) for r in results])
    return stack[plan["core_of"], plan["loc_of"], :]


def _wcat(W, att_src, att_dst, H, C):
    Wa_s = np.stack([W[:, h * C:(h + 1) * C] @ att_src[h] for h in range(H)], 1)
    Wa_d = np.stack([W[:, h * C:(h + 1) * C] @ att_dst[h] for h in range(H)], 1)
    return np.concatenate([W, Wa_s, Wa_d], axis=1).astype(np.float32)


def kernel(x, edge_index, edge_weight, W1, att_src1, att_dst1, W_edge1,
           att_edge1, b1, W2, att_src2, att_dst2, W_edge2, att_edge2, b2):
    global LAST_EXEC_NS
    LAST_EXEC_NS = []
    trace = os.environ.get("BASSGNN_TRACE", "0") == "1"

    x = np.asarray(x, np.float32)
    ew = np.asarray(edge_weight, np.float32).reshape(-1)
    plan = _plan(np.asarray(edge_index))
    core_ids = list(range(NCORES))

    k1 = np.array([W_edge1[0, h * 64:(h + 1) * 64] @ att_edge1[h]
                   for h in range(2)], np.float32)
    k2 = np.array([W_edge2[0, :64] @ att_edge2[0]], np.float32)
    W1c = _wcat(np.asarray(W1, np.float32), np.asarray(att_src1),
                np.asarray(att_dst1), 2, 64)
    W2c = _wcat(np.asarray(W2, np.float32), np.asarray(att_src2),
                np.asarray(att_dst2), 1, 64)

    # ---- P1 ----
    order = plan["order"]
    xT = np.ascontiguousarray(x.T).astype(NPBF)
    nc1 = _build_proj(132, 128)
    maps1 = []
    for c in range(NCORES):
        nodes = order[c::NCORES]
        xTc = np.zeros((P, NROWS), NPBF)
        xTc[:, :nodes.size] = xT[:, nodes]
        maps1.append({"xT": xTc, "wcat": W1c.astype(NPBF)})
    r1 = run_bass_kernel_spmd(nc1, maps1, core_ids, trace=trace)
    if trace:
        LAST_EXEC_NS.append(r1.exec_time_ns)
    proj1 = _collect(plan, r1.results, "hb")             # [N, 132] f32
    h1b = proj1[:, :128].astype(NPBF)
    att1 = proj1[:, 128:].astype(np.float32)

    # ---- P2 ----
    nc2 = _build_agg(plan["KT"], 2, 64, relu=True, proj_cols=66)
    maps2 = _gather_inputs(plan, h1b, att1, ew, k1, np.asarray(b1), 2, 64,
                           w2c=W2c.astype(NPBF))
    r2 = run_bass_kernel_spmd(nc2, maps2, core_ids, trace=trace)
    if trace:
        LAST_EXEC_NS.append(r2.exec_time_ns)
    h2 = _collect(plan, r2.results, "out")
    h2b = h2[:, :64].astype(NPBF)
    att2 = h2[:, 64:66].astype(np.float32)

    # ---- P3 ----
    nc3 = _build_agg(plan["KT"], 1, 64, relu=False, proj_cols=0)
    maps3 = _gather_inputs(plan, h2b, att2, ew, k2, np.asarray(b2), 1, 64)
    r3 = run_bass_kernel_spmd(nc3, maps3, core_ids, trace=trace)
    if trace:
        LAST_EXEC_NS.append(r3.exec_time_ns)
    return _collect(plan, r3.results, "out").astype(np.float32)
